# revision 15
# baseline (speedup 1.0000x reference)
"""CrossAttentionMemory kernel for 8 Trainium2 NeuronCores.

Reference computation (B=2, N=512, M=2048, D=H=4096):
    xq = inputs @ wq^T            [B, N, H]
    mk = memory @ wk^T            [B, M, H]
    s  = (xq @ mk^T) / sqrt(H)    [B, N, M]
    p  = softmax(s, f32) -> bf16
    out = p @ memory              [B, N, D]
    hist = seg_num - argmax(p, axis=2)  (flattened), browse = hist[0] < 4

Sharding: core c handles batch b=c//4 and memory rows j*512:(j+1)*512 (j=c%4).
Each core computes its local raw scores [512, 512] (f32) and the local
exp-weighted partial output sum [512, 4096] (f32, stabilized with the local
row max). The host combines shards flash-style and computes hist/browse from
the gathered f32 scores.

All device matmuls use contraction-major operands produced by host-side
numpy transposes (inputs.T, memory_slice.T, wq.T, wk.T), so no DMA
transposes are needed; the only on-device transposes are cheap 128x128 PE
transposes of intermediates (xq, mk, p).
"""

import sys

if "/opt/trn_rl_repo" not in sys.path:
    sys.path.insert(0, "/opt/trn_rl_repo")

import numpy as np
import ml_dtypes

B = 2
N = 512
M = 2048
D = 4096
H = 4096
P = 128
MLOC = M // 4  # 512 memory rows per core
NCORES = 8

BF16 = ml_dtypes.bfloat16

_CACHE = {}


def _build_program():
    import concourse.bacc as bacc
    import concourse.mybir as mybir
    import concourse.tile as tile
    from concourse.masks import make_identity

    fp32 = mybir.dt.float32
    bf16 = mybir.dt.bfloat16

    nc = bacc.Bacc("TRN2", target_bir_lowering=False)

    KC_ = D // P
    HS_ = H // 512
    # per-core slice of inputs^T: this core's 128 query rows (its batch
    # group AllGathers the four pieces to reassemble the full xqT)
    xin_t = nc.dram_tensor("xin_t", [D, P], bf16, kind="ExternalInput")
    mem_t = nc.dram_tensor("mem_t", [D, MLOC], bf16, kind="ExternalInput")
    mem_n = nc.dram_tensor("mem_n", [MLOC, D], bf16, kind="ExternalInput")
    # weights pre-tiled on host: [hs, k, 128, 512] so each (hs, k) weight
    # tile is one contiguous 128KB DMA
    wq_t = nc.dram_tensor("wq_t", [HS_, KC_, P, 512], bf16, kind="ExternalInput")
    wk_t = nc.dram_tensor("wk_t", [HS_, KC_, P, 512], bf16, kind="ExternalInput")

    out_part = nc.dram_tensor("out_part", [N, D], fp32, kind="ExternalOutput")
    scores_raw = nc.dram_tensor("scores_raw", [N, MLOC], fp32, kind="ExternalOutput")

    KC = D // P  # 32 contraction chunks for the projections
    HS = H // 512  # 8 output column slices for the projections
    NCH = N // P  # 4 query-row chunks
    MCH = MLOC // P  # 4 memory-row chunks
    DS = D // 512  # 8 output column slices for the final matmul

    groups = [[0, 1, 2, 3], [4, 5, 6, 7]]

    with tile.TileContext(nc) as tc:
        with (
            tc.tile_pool(name="const", bufs=1) as const,
            tc.tile_pool(name="act_in", bufs=1) as act_in,
            tc.tile_pool(name="wstream", bufs=6) as wstream,
            tc.tile_pool(name="psum_acc", bufs=6, space="PSUM") as psum_acc,
            tc.tile_pool(name="psum_tr", bufs=2, space="PSUM") as psum_tr,
            tc.tile_pool(name="evict", bufs=4) as evict,
            tc.tile_pool(name="hmajor", bufs=1) as hmajor,
            tc.tile_pool(name="dram", bufs=1, space="DRAM") as dram,
            tc.tile_pool(name="soft", bufs=3) as soft,
            tc.tile_pool(name="stats", bufs=8) as stats,
            tc.tile_pool(name="outev", bufs=4) as outev,
        ):
            ident = const.tile([P, P], bf16)
            make_identity(nc, ident)

            # resident activation inputs (contraction-major)
            xin_sb = act_in.tile([P, KC, P], bf16, tag="xin")
            memt_sb = act_in.tile([P, KC, MLOC], bf16, tag="memt")
            memn_sb = act_in.tile([P, MCH, D], bf16, tag="memn")
            for k in range(KC):
                nc.sync.dma_start(out=xin_sb[:, k, :], in_=xin_t[k * P:(k + 1) * P, :])

            # H-major projected activations
            xqT = hmajor.tile([P, KC, N], bf16, tag="xqT")
            xqT_loc = hmajor.tile([P, KC, P], bf16, tag="xqT_loc")
            mkT = hmajor.tile([P, KC, MLOC], bf16, tag="mkT")

            def project_hs(w_dram, act_sb, nch, hs, write_block):
                psums = [
                    psum_acc.tile([P, 512], fp32, tag="pacc", name="pacc")
                    for _ in range(nch)
                ]
                for k in range(KC):
                    wt = wstream.tile([P, 512], bf16, tag="wt", name="wt")
                    nc.sync.dma_start(out=wt[:], in_=w_dram[hs, k])
                    for ni in range(nch):
                        nc.tensor.matmul(
                            psums[ni][:],
                            lhsT=act_sb[:, k, ni * P:(ni + 1) * P],
                            rhs=wt[:],
                            start=(k == 0),
                            stop=(k == KC - 1),
                        )
                for ni in range(nch):
                    ev = evict.tile([P, 512], bf16, tag="ev", name="ev")
                    nc.scalar.copy(ev[:], psums[ni][:])
                    for t in range(4):
                        pt = psum_tr.tile([P, P], bf16, tag="pt", name="pt")
                        nc.tensor.transpose(
                            pt[:], ev[:, t * P:(t + 1) * P], ident[:]
                        )
                        write_block(hs * 4 + t, ni, pt)

            def write_xq(h_chunk, ni, pt):
                nc.vector.tensor_copy(xqT_loc[:, h_chunk, :], pt[:])

            def write_mk(h_chunk, ni, pt):
                nc.vector.tensor_copy(mkT[:, h_chunk, ni * P:(ni + 1) * P], pt[:])

            # interleave: front-load the (cheap) local xq slices between mk
            # slices so the AllGather can start early and hide under the
            # remaining mk projection work
            order = []
            xq_left, mk_left = list(range(HS)), list(range(HS))
            while xq_left or mk_left:
                for _ in range(2):
                    if xq_left:
                        order.append(("xq", xq_left.pop(0)))
                if mk_left:
                    order.append(("mk", mk_left.pop(0)))
            emitted_memt = False
            for kind_, hs in order:
                if kind_ == "xq":
                    project_hs(wq_t, xin_sb, 1, hs, write_xq)
                else:
                    project_hs(wk_t, memt_sb, 4, hs, write_mk)
                if not emitted_memt:
                    # queue the memt load right after the first weight
                    # slice so the first mk matmuls aren't starved
                    for k in range(KC):
                        nc.sync.dma_start(
                            out=memt_sb[:, k, :], in_=mem_t[k * P:(k + 1) * P, :]
                        )
                    emitted_memt = True

            # AllGather the four xqT quarters within each batch group
            piece = dram.tile([P, KC, P], bf16, tag="piece")
            gathered = dram.tile([4, P, KC, P], bf16, tag="gathered")
            nc.sync.dma_start(out=piece[:], in_=xqT_loc[:])
            nc.gpsimd.collective_compute(
                "AllGather",
                mybir.AluOpType.bypass,
                replica_groups=groups,
                ins=[piece[:]],
                outs=[gathered[:]],
            )
            for j in range(4):
                nc.sync.dma_start(
                    out=xqT[:, :, j * P:(j + 1) * P], in_=gathered[j]
                )

            for mc in range(MCH):
                nc.sync.dma_start(
                    out=memn_sb[:, mc, :], in_=mem_n[mc * P:(mc + 1) * P, :]
                )

            # scores + softmax numerator + p^T
            pT = hmajor.tile([P, MCH, N], bf16, tag="pT")
            for ni in range(NCH):
                ps = psum_acc.tile([P, MLOC], fp32, tag="pacc")
                for k in range(KC):
                    nc.tensor.matmul(
                        ps[:],
                        lhsT=xqT[:, k, ni * P:(ni + 1) * P],
                        rhs=mkT[:, k, :],
                        start=(k == 0),
                        stop=(k == KC - 1),
                    )
                # The reference materializes the scores einsum in bf16, so
                # quantize raw scores to bf16 before the softmax chain; the
                # host performs the identical quantization on scores_raw.
                sc = soft.tile([P, MLOC], fp32, tag="sc")
                nc.vector.tensor_copy(sc[:], ps[:])
                nc.sync.dma_start(
                    out=scores_raw[ni * P:(ni + 1) * P, :], in_=sc[:]
                )
                sb = soft.tile([P, MLOC], bf16, tag="sb")
                nc.vector.tensor_copy(sb[:], ps[:])
                mx = stats.tile([P, 1], fp32, tag="mx")
                nc.vector.reduce_max(mx[:], sb[:], axis=mybir.AxisListType.X)
                nb = stats.tile([P, 1], fp32, tag="nb")
                nc.scalar.mul(nb[:], mx[:], -1.0 / 64.0)
                pb = soft.tile([P, MLOC], bf16, tag="pb")
                nc.scalar.activation(
                    pb[:],
                    sb[:],
                    mybir.ActivationFunctionType.Exp,
                    bias=nb[:],
                    scale=1.0 / 64.0,
                )
                for mc in range(MCH):
                    pt = psum_tr.tile([P, P], bf16)
                    nc.tensor.transpose(
                        pt[:], pb[:, mc * P:(mc + 1) * P], ident[:]
                    )
                    nc.vector.tensor_copy(
                        pT[:, mc, ni * P:(ni + 1) * P], pt[:]
                    )

            # out_part = p @ mem_slice (f32 partials)
            for ni in range(NCH):
                for dsi in range(DS):
                    po = psum_acc.tile([P, 512], fp32, tag="pacc")
                    for mc in range(MCH):
                        nc.tensor.matmul(
                            po[:],
                            lhsT=pT[:, mc, ni * P:(ni + 1) * P],
                            rhs=memn_sb[:, mc, dsi * 512:(dsi + 1) * 512],
                            start=(mc == 0),
                            stop=(mc == MCH - 1),
                        )
                    oe = outev.tile([P, 512], fp32)
                    nc.scalar.copy(oe[:], po[:])
                    nc.sync.dma_start(
                        out=out_part[ni * P:(ni + 1) * P, dsi * 512:(dsi + 1) * 512],
                        in_=oe[:],
                    )

    nc.compile()
    return nc


def _get_program():
    if "nc" not in _CACHE:
        _CACHE["nc"] = _build_program()
    return _CACHE["nc"]


def _as_bf16(x):
    x = np.asarray(x)
    if x.dtype != BF16:
        x = x.astype(BF16)
    return x


def make_in_maps(memory, inputs, wq, wk):
    memory = _as_bf16(memory)
    inputs = _as_bf16(inputs)
    wq = _as_bf16(wq)
    wk = _as_bf16(wk)

    def retile_w(w):
        # w is [H, D]; device wants w.T = [D, H] pre-tiled as
        # [hs, k, 128, 512] with contiguous (hs, k) tiles
        wt = np.ascontiguousarray(w.T).reshape(D // P, P, H // 512, 512)
        return np.ascontiguousarray(wt.transpose(2, 0, 1, 3))

    wq_t = retile_w(wq)
    wk_t = retile_w(wk)
    in_maps = []
    for c in range(NCORES):
        b, j = divmod(c, 4)
        mem_slice = memory[b, j * MLOC:(j + 1) * MLOC]
        in_maps.append(
            {
                "xin_t": np.ascontiguousarray(inputs[b].T[:, j * P:(j + 1) * P]),
                "mem_t": np.ascontiguousarray(mem_slice.T),
                "mem_n": np.ascontiguousarray(mem_slice),
                "wq_t": wq_t,
                "wk_t": wk_t,
            }
        )
    return in_maps


def kernel(memory, inputs, wq, wk, seg_num, _want_results=False):
    from concourse.bass_utils import run_bass_kernel_spmd

    seg = int(np.asarray(seg_num))
    nc = _get_program()
    in_maps = make_in_maps(memory, inputs, wq, wk)
    res = run_bass_kernel_spmd(nc, in_maps, list(range(NCORES)))

    output = np.empty((B, N, D), dtype=np.float32)
    hist_parts = []
    for b in range(B):
        raw = np.concatenate(
            [res.results[4 * b + j]["scores_raw"] for j in range(4)], axis=1
        ).astype(np.float32)  # [N, M]
        # mirror the reference: the scores einsum materializes in bf16
        logits = (raw * np.float32(1.0 / 64.0)).astype(BF16).astype(np.float32)
        m_glob = logits.max(axis=1)
        l_glob = np.exp(logits - m_glob[:, None]).sum(axis=1)
        acc = np.zeros((N, D), dtype=np.float32)
        for j in range(4):
            m_loc = logits[:, j * MLOC:(j + 1) * MLOC].max(axis=1)
            scale = np.exp(m_loc - m_glob)
            acc += res.results[4 * b + j]["out_part"] * scale[:, None]
        output[b] = acc / l_glob[:, None]
        am = np.argmax(logits, axis=1)
        hist_parts.append((seg - am).astype(np.int32))

    hist = np.concatenate(hist_parts)
    browse = np.bool_(hist[0] < 4)
    out = (output.astype(BF16), hist, browse)
    if _want_results:
        return out, res
    return out


# revision 22
# speedup vs baseline: 1.2643x; 1.2643x over previous
"""CrossAttentionMemory kernel for 8 Trainium2 NeuronCores.

Reference computation (B=2, N=512, M=2048, D=H=4096):
    xq = inputs @ wq^T            [B, N, H]
    mk = memory @ wk^T            [B, M, H]
    s  = (xq @ mk^T) / sqrt(H)    [B, N, M]
    p  = softmax(s, f32) -> bf16
    out = p @ memory              [B, N, D]
    hist = seg_num - argmax(p, axis=2)  (flattened), browse = hist[0] < 4

Sharding: core c handles batch b=c//4 and memory rows j*512:(j+1)*512 (j=c%4).
Each core computes its local raw scores [512, 512] (f32) and the local
exp-weighted partial output sum [512, 4096] (f32, stabilized with the local
row max). The host combines shards flash-style and computes hist/browse from
the gathered f32 scores.

All device matmuls use contraction-major operands produced by host-side
numpy transposes (inputs.T, memory_slice.T, wq.T, wk.T), so no DMA
transposes are needed; the only on-device transposes are cheap 128x128 PE
transposes of intermediates (xq, mk, p).
"""

import sys

if "/opt/trn_rl_repo" not in sys.path:
    sys.path.insert(0, "/opt/trn_rl_repo")

import numpy as np
import ml_dtypes

B = 2
N = 512
M = 2048
D = 4096
H = 4096
P = 128
MLOC = M // 4  # 512 memory rows per core
NCORES = 8

BF16 = ml_dtypes.bfloat16

_CACHE = {}


def _build_program():
    import concourse.bacc as bacc
    import concourse.mybir as mybir
    import concourse.tile as tile
    from concourse.masks import make_identity

    fp32 = mybir.dt.float32
    bf16 = mybir.dt.bfloat16

    nc = bacc.Bacc("TRN2", target_bir_lowering=False)

    KC_ = D // P
    HS_ = H // 512
    xin_t = nc.dram_tensor("xin_t", [D, N], bf16, kind="ExternalInput")
    mem_t = nc.dram_tensor("mem_t", [D, MLOC], bf16, kind="ExternalInput")
    mem_n = nc.dram_tensor("mem_n", [MLOC, D], bf16, kind="ExternalInput")
    # weights pre-tiled on host: [hs, k, 128, 512] so each (hs, k) weight
    # tile is one contiguous 128KB DMA. wq arrives as this core's 2-slice
    # share of the hidden dim (its batch group AllGathers the xq pieces).
    wq_t = nc.dram_tensor("wq_t", [2, KC_, P, 512], bf16, kind="ExternalInput")
    wk_t = nc.dram_tensor("wk_t", [HS_, KC_, P, 512], bf16, kind="ExternalInput")

    out_part = nc.dram_tensor("out_part", [N, D], fp32, kind="ExternalOutput")
    scores_raw = nc.dram_tensor("scores_raw", [N, MLOC], fp32, kind="ExternalOutput")

    KC = D // P  # 32 contraction chunks for the projections
    HS = H // 512  # 8 output column slices for the projections
    NCH = N // P  # 4 query-row chunks
    MCH = MLOC // P  # 4 memory-row chunks
    DS = D // 512  # 8 output column slices for the final matmul

    groups = [[0, 1, 2, 3], [4, 5, 6, 7]]

    with tile.TileContext(nc) as tc:
        with (
            tc.tile_pool(name="const", bufs=1) as const,
            tc.tile_pool(name="act_in", bufs=1) as act_in,
            tc.tile_pool(name="wstream", bufs=6) as wstream,
            tc.tile_pool(name="psum_acc", bufs=6, space="PSUM") as psum_acc,
            tc.tile_pool(name="psum_tr", bufs=2, space="PSUM") as psum_tr,
            tc.tile_pool(name="evict", bufs=4) as evict,
            tc.tile_pool(name="hmajor", bufs=1) as hmajor,
            tc.tile_pool(name="dram", bufs=1, space="DRAM") as dram,
            tc.tile_pool(name="soft", bufs=3) as soft,
            tc.tile_pool(name="stats", bufs=8) as stats,
            tc.tile_pool(name="outev", bufs=4) as outev,
        ):
            ident = const.tile([P, P], bf16)
            make_identity(nc, ident)

            # resident activation inputs (contraction-major)
            xin_sb = act_in.tile([P, KC, N], bf16, tag="xin")
            memt_sb = act_in.tile([P, KC, MLOC], bf16, tag="memt")
            memn_sb = act_in.tile([P, MCH, D], bf16, tag="memn")
            for k in range(KC):
                nc.sync.dma_start(out=xin_sb[:, k, :], in_=xin_t[k * P:(k + 1) * P, :])

            # H-major projected activations; xqT_loc holds this core's 8
            # h-chunks (of 32) for all N
            xqT = hmajor.tile([P, KC, N], bf16, tag="xqT")
            xqT_loc = hmajor.tile([P, 8, N], bf16, tag="xqT_loc")
            mkT = hmajor.tile([P, KC, MLOC], bf16, tag="mkT")

            def project_hs(w_dram, act_sb, nch, hs, write_block):
                psums = [
                    psum_acc.tile([P, 512], fp32, tag="pacc", name="pacc")
                    for _ in range(nch)
                ]
                for k in range(KC):
                    wt = wstream.tile([P, 512], bf16, tag="wt", name="wt")
                    nc.sync.dma_start(out=wt[:], in_=w_dram[hs, k])
                    for ni in range(nch):
                        nc.tensor.matmul(
                            psums[ni][:],
                            lhsT=act_sb[:, k, ni * P:(ni + 1) * P],
                            rhs=wt[:],
                            start=(k == 0),
                            stop=(k == KC - 1),
                        )
                for ni in range(nch):
                    ev = evict.tile([P, 512], bf16, tag="ev", name="ev")
                    nc.scalar.copy(ev[:], psums[ni][:])
                    for t in range(4):
                        pt = psum_tr.tile([P, P], bf16, tag="pt", name="pt")
                        nc.tensor.transpose(
                            pt[:], ev[:, t * P:(t + 1) * P], ident[:]
                        )
                        write_block(hs * 4 + t, ni, pt)

            def write_xq(h_chunk, ni, pt):
                nc.vector.tensor_copy(
                    xqT_loc[:, h_chunk, ni * P:(ni + 1) * P], pt[:]
                )

            def write_mk(h_chunk, ni, pt):
                nc.vector.tensor_copy(mkT[:, h_chunk, ni * P:(ni + 1) * P], pt[:])

            # local xq share first (2 of 8 hidden slices, full N), then the
            # AllGather runs while the mk projection keeps PE busy
            for hs in range(2):
                project_hs(wq_t, xin_sb, 4, hs, write_xq)

            piece = dram.tile([P, 8, N], bf16, tag="piece")
            gathered = dram.tile([4, P, 8, N], bf16, tag="gathered")
            nc.sync.dma_start(out=piece[:], in_=xqT_loc[:])
            nc.gpsimd.collective_compute(
                "AllGather",
                mybir.AluOpType.bypass,
                replica_groups=groups,
                ins=[piece[:]],
                outs=[gathered[:]],
            )
            for k in range(KC):
                nc.sync.dma_start(
                    out=memt_sb[:, k, :], in_=mem_t[k * P:(k + 1) * P, :]
                )
            for hs in range(HS):
                project_hs(wk_t, memt_sb, 4, hs, write_mk)
                if hs == 2:
                    # pull the gathered xq pieces back once the collective
                    # has certainly finished; placed mid-stream so these
                    # DMAs never head-block the memt/weight queues
                    for j in range(4):
                        nc.sync.dma_start(
                            out=xqT[:, j * 8:(j + 1) * 8, :], in_=gathered[j]
                        )

            for mc in range(MCH):
                nc.sync.dma_start(
                    out=memn_sb[:, mc, :], in_=mem_n[mc * P:(mc + 1) * P, :]
                )

            # scores + softmax numerator + p^T
            pT = hmajor.tile([P, MCH, N], bf16, tag="pT")
            for ni in range(NCH):
                ps = psum_acc.tile([P, MLOC], fp32, tag="pacc")
                for k in range(KC):
                    nc.tensor.matmul(
                        ps[:],
                        lhsT=xqT[:, k, ni * P:(ni + 1) * P],
                        rhs=mkT[:, k, :],
                        start=(k == 0),
                        stop=(k == KC - 1),
                    )
                # The reference materializes the scores einsum in bf16, so
                # quantize raw scores to bf16 before the softmax chain; the
                # host performs the identical quantization on scores_raw.
                sc = soft.tile([P, MLOC], fp32, tag="sc")
                nc.vector.tensor_copy(sc[:], ps[:])
                nc.sync.dma_start(
                    out=scores_raw[ni * P:(ni + 1) * P, :], in_=sc[:]
                )
                sb = soft.tile([P, MLOC], bf16, tag="sb")
                nc.vector.tensor_copy(sb[:], ps[:])
                mx = stats.tile([P, 1], fp32, tag="mx")
                nc.vector.reduce_max(mx[:], sb[:], axis=mybir.AxisListType.X)
                nb = stats.tile([P, 1], fp32, tag="nb")
                nc.scalar.mul(nb[:], mx[:], -1.0 / 64.0)
                pb = soft.tile([P, MLOC], bf16, tag="pb")
                nc.scalar.activation(
                    pb[:],
                    sb[:],
                    mybir.ActivationFunctionType.Exp,
                    bias=nb[:],
                    scale=1.0 / 64.0,
                )
                for mc in range(MCH):
                    pt = psum_tr.tile([P, P], bf16)
                    nc.tensor.transpose(
                        pt[:], pb[:, mc * P:(mc + 1) * P], ident[:]
                    )
                    nc.vector.tensor_copy(
                        pT[:, mc, ni * P:(ni + 1) * P], pt[:]
                    )

            # out_part = p @ mem_slice (f32 partials)
            for ni in range(NCH):
                for dsi in range(DS):
                    po = psum_acc.tile([P, 512], fp32, tag="pacc")
                    for mc in range(MCH):
                        nc.tensor.matmul(
                            po[:],
                            lhsT=pT[:, mc, ni * P:(ni + 1) * P],
                            rhs=memn_sb[:, mc, dsi * 512:(dsi + 1) * 512],
                            start=(mc == 0),
                            stop=(mc == MCH - 1),
                        )
                    oe = outev.tile([P, 512], fp32)
                    nc.scalar.copy(oe[:], po[:])
                    nc.sync.dma_start(
                        out=out_part[ni * P:(ni + 1) * P, dsi * 512:(dsi + 1) * 512],
                        in_=oe[:],
                    )

    nc.compile()
    return nc


def _get_program():
    if "nc" not in _CACHE:
        _CACHE["nc"] = _build_program()
    return _CACHE["nc"]


def _as_bf16(x):
    x = np.asarray(x)
    if x.dtype != BF16:
        x = x.astype(BF16)
    return x


def make_in_maps(memory, inputs, wq, wk):
    memory = _as_bf16(memory)
    inputs = _as_bf16(inputs)
    wq = _as_bf16(wq)
    wk = _as_bf16(wk)

    def retile_w(w):
        # w is [H, D]; device wants w.T = [D, H] pre-tiled as
        # [hs, k, 128, 512] with contiguous (hs, k) tiles
        wt = np.ascontiguousarray(w.T).reshape(D // P, P, H // 512, 512)
        return np.ascontiguousarray(wt.transpose(2, 0, 1, 3))

    wq_t = retile_w(wq)
    wk_t = retile_w(wk)
    in_maps = []
    for c in range(NCORES):
        b, j = divmod(c, 4)
        mem_slice = memory[b, j * MLOC:(j + 1) * MLOC]
        in_maps.append(
            {
                "xin_t": np.ascontiguousarray(inputs[b].T),
                "wq_t": np.ascontiguousarray(wq_t[2 * j:2 * j + 2]),
                "mem_t": np.ascontiguousarray(mem_slice.T),
                "mem_n": np.ascontiguousarray(mem_slice),
                "wk_t": wk_t,
            }
        )
    return in_maps


def kernel(memory, inputs, wq, wk, seg_num, _want_results=False):
    from concourse.bass_utils import run_bass_kernel_spmd

    seg = int(np.asarray(seg_num))
    nc = _get_program()
    in_maps = make_in_maps(memory, inputs, wq, wk)
    res = run_bass_kernel_spmd(nc, in_maps, list(range(NCORES)))

    output = np.empty((B, N, D), dtype=np.float32)
    hist_parts = []
    for b in range(B):
        raw = np.concatenate(
            [res.results[4 * b + j]["scores_raw"] for j in range(4)], axis=1
        ).astype(np.float32)  # [N, M]
        # mirror the reference: the scores einsum materializes in bf16
        logits = (raw * np.float32(1.0 / 64.0)).astype(BF16).astype(np.float32)
        m_glob = logits.max(axis=1)
        l_glob = np.exp(logits - m_glob[:, None]).sum(axis=1)
        acc = np.zeros((N, D), dtype=np.float32)
        for j in range(4):
            m_loc = logits[:, j * MLOC:(j + 1) * MLOC].max(axis=1)
            scale = np.exp(m_loc - m_glob)
            acc += res.results[4 * b + j]["out_part"] * scale[:, None]
        output[b] = acc / l_glob[:, None]
        am = np.argmax(logits, axis=1)
        hist_parts.append((seg - am).astype(np.int32))

    hist = np.concatenate(hist_parts)
    browse = np.bool_(hist[0] < 4)
    out = (output.astype(BF16), hist, browse)
    if _want_results:
        return out, res
    return out


# revision 25
# speedup vs baseline: 1.3224x; 1.0460x over previous
"""CrossAttentionMemory kernel for 8 Trainium2 NeuronCores.

Reference computation (B=2, N=512, M=2048, D=H=4096):
    xq = inputs @ wq^T            [B, N, H]
    mk = memory @ wk^T            [B, M, H]
    s  = (xq @ mk^T) / sqrt(H)    [B, N, M]
    p  = softmax(s, f32) -> bf16
    out = p @ memory              [B, N, D]
    hist = seg_num - argmax(p, axis=2)  (flattened), browse = hist[0] < 4

Sharding: core c handles batch b=c//4 and memory rows j*512:(j+1)*512 (j=c%4).
Each core computes its local raw scores [512, 512] (f32) and the local
exp-weighted partial output sum [512, 4096] (f32, stabilized with the local
row max). The host combines shards flash-style and computes hist/browse from
the gathered f32 scores.

All device matmuls use contraction-major operands produced by host-side
numpy transposes (inputs.T, memory_slice.T, wq.T, wk.T), so no DMA
transposes are needed; the only on-device transposes are cheap 128x128 PE
transposes of intermediates (xq, mk, p).
"""

import sys

if "/opt/trn_rl_repo" not in sys.path:
    sys.path.insert(0, "/opt/trn_rl_repo")

import numpy as np
import ml_dtypes

B = 2
N = 512
M = 2048
D = 4096
H = 4096
P = 128
MLOC = M // 4  # 512 memory rows per core
NCORES = 8

BF16 = ml_dtypes.bfloat16

_CACHE = {}


def _build_program():
    import concourse.bacc as bacc
    import concourse.mybir as mybir
    import concourse.tile as tile
    from concourse.masks import make_identity

    fp32 = mybir.dt.float32
    bf16 = mybir.dt.bfloat16

    nc = bacc.Bacc("TRN2", target_bir_lowering=False)

    KC_ = D // P
    HS_ = H // 512
    xin_t = nc.dram_tensor("xin_t", [D, N], bf16, kind="ExternalInput")
    mem_t = nc.dram_tensor("mem_t", [D, MLOC], bf16, kind="ExternalInput")
    mem_n = nc.dram_tensor("mem_n", [MLOC, D], bf16, kind="ExternalInput")
    # weights pre-tiled on host: [hs, k, 128, 512] so each (hs, k) weight
    # tile is one contiguous 128KB DMA. wq arrives as this core's 2-slice
    # share of the hidden dim (its batch group AllGathers the xq pieces).
    wq_t = nc.dram_tensor("wq_t", [2, KC_, P, 512], bf16, kind="ExternalInput")
    wk_t = nc.dram_tensor("wk_t", [HS_, KC_, P, 512], bf16, kind="ExternalInput")

    out_part = nc.dram_tensor("out_part", [N, D], fp32, kind="ExternalOutput")
    scores_raw = nc.dram_tensor("scores_raw", [N, MLOC], fp32, kind="ExternalOutput")

    KC = D // P  # 32 contraction chunks for the projections
    HS = H // 512  # 8 output column slices for the projections
    NCH = N // P  # 4 query-row chunks
    MCH = MLOC // P  # 4 memory-row chunks
    DS = D // 512  # 8 output column slices for the final matmul

    groups = [[0, 1, 2, 3], [4, 5, 6, 7]]

    with tile.TileContext(nc) as tc:
        with (
            tc.tile_pool(name="const", bufs=1) as const,
            tc.tile_pool(name="act_in", bufs=1) as act_in,
            tc.tile_pool(name="wstream", bufs=6) as wstream,
            tc.tile_pool(name="psum_acc", bufs=6, space="PSUM") as psum_acc,
            tc.tile_pool(name="psum_tr", bufs=2, space="PSUM") as psum_tr,
            tc.tile_pool(name="evict", bufs=4) as evict,
            tc.tile_pool(name="hmajor", bufs=1) as hmajor,
            tc.tile_pool(name="dram", bufs=1, space="DRAM") as dram,
            tc.tile_pool(name="soft", bufs=3) as soft,
            tc.tile_pool(name="stats", bufs=8) as stats,
            tc.tile_pool(name="outev", bufs=4) as outev,
        ):
            ident = const.tile([P, P], bf16)
            make_identity(nc, ident)

            # resident activation inputs (contraction-major); one tile per
            # d-chunk so the first matmuls only wait on the chunks they read
            xin_sb = [
                act_in.tile([P, N], bf16, tag=f"xin{k}", name=f"xin{k}")
                for k in range(KC)
            ]
            memt_sb = [
                act_in.tile([P, MLOC], bf16, tag=f"memt{k}", name=f"memt{k}")
                for k in range(KC)
            ]
            memn_sb = act_in.tile([P, MCH, D], bf16, tag="memn")
            for k in range(KC):
                nc.sync.dma_start(out=xin_sb[k][:], in_=xin_t[k * P:(k + 1) * P, :])

            # H-major projected activations; xqT_loc holds this core's 8
            # h-chunks (of 32) for all N
            xqT = hmajor.tile([P, KC, N], bf16, tag="xqT")
            xqT_loc = hmajor.tile([P, 8, N], bf16, tag="xqT_loc")
            mkT = hmajor.tile([P, KC, MLOC], bf16, tag="mkT")

            def project_hs(w_dram, act_sb, nch, hs, write_block):
                psums = [
                    psum_acc.tile([P, 512], fp32, tag="pacc", name="pacc")
                    for _ in range(nch)
                ]
                for k in range(KC):
                    wt = wstream.tile([P, 512], bf16, tag="wt", name="wt")
                    nc.sync.dma_start(out=wt[:], in_=w_dram[hs, k])
                    for ni in range(nch):
                        nc.tensor.matmul(
                            psums[ni][:],
                            lhsT=act_sb[k][:, ni * P:(ni + 1) * P],
                            rhs=wt[:],
                            start=(k == 0),
                            stop=(k == KC - 1),
                        )
                for ni in range(nch):
                    ev = evict.tile([P, 512], bf16, tag="ev", name="ev")
                    nc.scalar.copy(ev[:], psums[ni][:])
                    for t in range(4):
                        pt = psum_tr.tile([P, P], bf16, tag="pt", name="pt")
                        nc.tensor.transpose(
                            pt[:], ev[:, t * P:(t + 1) * P], ident[:]
                        )
                        write_block(hs * 4 + t, ni, pt)

            def write_xq(h_chunk, ni, pt):
                nc.vector.tensor_copy(
                    xqT_loc[:, h_chunk, ni * P:(ni + 1) * P], pt[:]
                )

            def write_mk(h_chunk, ni, pt):
                nc.vector.tensor_copy(mkT[:, h_chunk, ni * P:(ni + 1) * P], pt[:])

            # local xq share first (2 of 8 hidden slices, full N), then the
            # AllGather runs while the mk projection keeps PE busy
            for hs in range(2):
                project_hs(wq_t, xin_sb, 4, hs, write_xq)

            piece = dram.tile([P, 8, N], bf16, tag="piece")
            gathered = dram.tile([4, P, 8, N], bf16, tag="gathered")
            nc.sync.dma_start(out=piece[:], in_=xqT_loc[:])
            nc.gpsimd.collective_compute(
                "AllGather",
                mybir.AluOpType.bypass,
                replica_groups=groups,
                ins=[piece[:]],
                outs=[gathered[:]],
            )
            for k in range(KC):
                nc.sync.dma_start(
                    out=memt_sb[k][:], in_=mem_t[k * P:(k + 1) * P, :]
                )
            for hs in range(HS):
                project_hs(wk_t, memt_sb, 4, hs, write_mk)
            # pull the gathered xq pieces back after the whole weight
            # stream is queued: these DMAs wait on the collective, and
            # anything queued behind them would head-block its DMA queue
            for j in range(4):
                nc.sync.dma_start(
                    out=xqT[:, j * 8:(j + 1) * 8, :], in_=gathered[j]
                )

            for mc in range(MCH):
                nc.sync.dma_start(
                    out=memn_sb[:, mc, :], in_=mem_n[mc * P:(mc + 1) * P, :]
                )

            # scores + softmax numerator + p^T
            pT = hmajor.tile([P, MCH, N], bf16, tag="pT")
            for ni in range(NCH):
                ps = psum_acc.tile([P, MLOC], fp32, tag="pacc")
                for k in range(KC):
                    nc.tensor.matmul(
                        ps[:],
                        lhsT=xqT[:, k, ni * P:(ni + 1) * P],
                        rhs=mkT[:, k, :],
                        start=(k == 0),
                        stop=(k == KC - 1),
                    )
                # The reference materializes the scores einsum in bf16, so
                # quantize raw scores to bf16 before the softmax chain; the
                # host performs the identical quantization on scores_raw.
                sc = soft.tile([P, MLOC], fp32, tag="sc")
                nc.vector.tensor_copy(sc[:], ps[:])
                nc.sync.dma_start(
                    out=scores_raw[ni * P:(ni + 1) * P, :], in_=sc[:]
                )
                sb = soft.tile([P, MLOC], bf16, tag="sb")
                nc.vector.tensor_copy(sb[:], ps[:])
                mx = stats.tile([P, 1], fp32, tag="mx")
                nc.vector.reduce_max(mx[:], sb[:], axis=mybir.AxisListType.X)
                nb = stats.tile([P, 1], fp32, tag="nb")
                nc.scalar.mul(nb[:], mx[:], -1.0 / 64.0)
                pb = soft.tile([P, MLOC], bf16, tag="pb")
                nc.scalar.activation(
                    pb[:],
                    sb[:],
                    mybir.ActivationFunctionType.Exp,
                    bias=nb[:],
                    scale=1.0 / 64.0,
                )
                for mc in range(MCH):
                    pt = psum_tr.tile([P, P], bf16)
                    nc.tensor.transpose(
                        pt[:], pb[:, mc * P:(mc + 1) * P], ident[:]
                    )
                    nc.vector.tensor_copy(
                        pT[:, mc, ni * P:(ni + 1) * P], pt[:]
                    )

            # out_part = p @ mem_slice (f32 partials)
            for ni in range(NCH):
                for dsi in range(DS):
                    po = psum_acc.tile([P, 512], fp32, tag="pacc")
                    for mc in range(MCH):
                        nc.tensor.matmul(
                            po[:],
                            lhsT=pT[:, mc, ni * P:(ni + 1) * P],
                            rhs=memn_sb[:, mc, dsi * 512:(dsi + 1) * 512],
                            start=(mc == 0),
                            stop=(mc == MCH - 1),
                        )
                    oe = outev.tile([P, 512], fp32)
                    nc.scalar.copy(oe[:], po[:])
                    nc.sync.dma_start(
                        out=out_part[ni * P:(ni + 1) * P, dsi * 512:(dsi + 1) * 512],
                        in_=oe[:],
                    )

    nc.compile()
    return nc


def _get_program():
    if "nc" not in _CACHE:
        _CACHE["nc"] = _build_program()
    return _CACHE["nc"]


def _as_bf16(x):
    x = np.asarray(x)
    if x.dtype != BF16:
        x = x.astype(BF16)
    return x


def make_in_maps(memory, inputs, wq, wk):
    memory = _as_bf16(memory)
    inputs = _as_bf16(inputs)
    wq = _as_bf16(wq)
    wk = _as_bf16(wk)

    def retile_w(w):
        # w is [H, D]; device wants w.T = [D, H] pre-tiled as
        # [hs, k, 128, 512] with contiguous (hs, k) tiles
        wt = np.ascontiguousarray(w.T).reshape(D // P, P, H // 512, 512)
        return np.ascontiguousarray(wt.transpose(2, 0, 1, 3))

    wq_t = retile_w(wq)
    wk_t = retile_w(wk)
    in_maps = []
    for c in range(NCORES):
        b, j = divmod(c, 4)
        mem_slice = memory[b, j * MLOC:(j + 1) * MLOC]
        in_maps.append(
            {
                "xin_t": np.ascontiguousarray(inputs[b].T),
                "wq_t": np.ascontiguousarray(wq_t[2 * j:2 * j + 2]),
                "mem_t": np.ascontiguousarray(mem_slice.T),
                "mem_n": np.ascontiguousarray(mem_slice),
                "wk_t": wk_t,
            }
        )
    return in_maps


def kernel(memory, inputs, wq, wk, seg_num, _want_results=False):
    from concourse.bass_utils import run_bass_kernel_spmd

    seg = int(np.asarray(seg_num))
    nc = _get_program()
    in_maps = make_in_maps(memory, inputs, wq, wk)
    res = run_bass_kernel_spmd(nc, in_maps, list(range(NCORES)))

    output = np.empty((B, N, D), dtype=np.float32)
    hist_parts = []
    for b in range(B):
        raw = np.concatenate(
            [res.results[4 * b + j]["scores_raw"] for j in range(4)], axis=1
        ).astype(np.float32)  # [N, M]
        # mirror the reference: the scores einsum materializes in bf16
        logits = (raw * np.float32(1.0 / 64.0)).astype(BF16).astype(np.float32)
        m_glob = logits.max(axis=1)
        l_glob = np.exp(logits - m_glob[:, None]).sum(axis=1)
        acc = np.zeros((N, D), dtype=np.float32)
        for j in range(4):
            m_loc = logits[:, j * MLOC:(j + 1) * MLOC].max(axis=1)
            scale = np.exp(m_loc - m_glob)
            acc += res.results[4 * b + j]["out_part"] * scale[:, None]
        output[b] = acc / l_glob[:, None]
        am = np.argmax(logits, axis=1)
        hist_parts.append((seg - am).astype(np.int32))

    hist = np.concatenate(hist_parts)
    browse = np.bool_(hist[0] < 4)
    out = (output.astype(BF16), hist, browse)
    if _want_results:
        return out, res
    return out


# revision 32
# speedup vs baseline: 1.3298x; 1.0056x over previous
"""CrossAttentionMemory kernel for 8 Trainium2 NeuronCores.

Reference computation (B=2, N=512, M=2048, D=H=4096):
    xq = inputs @ wq^T            [B, N, H]
    mk = memory @ wk^T            [B, M, H]
    s  = (xq @ mk^T) / sqrt(H)    [B, N, M]
    p  = softmax(s, f32) -> bf16
    out = p @ memory              [B, N, D]
    hist = seg_num - argmax(p, axis=2)  (flattened), browse = hist[0] < 4

Sharding: core c handles batch b=c//4 and memory rows j*512:(j+1)*512 (j=c%4).
Each core computes its local raw scores [512, 512] (f32) and the local
exp-weighted partial output sum [512, 4096] (f32, stabilized with the local
row max). The host combines shards flash-style and computes hist/browse from
the gathered f32 scores.

All device matmuls use contraction-major operands produced by host-side
numpy transposes (inputs.T, memory_slice.T, wq.T, wk.T), so no DMA
transposes are needed; the only on-device transposes are cheap 128x128 PE
transposes of intermediates (xq, mk, p).
"""

import sys

if "/opt/trn_rl_repo" not in sys.path:
    sys.path.insert(0, "/opt/trn_rl_repo")

import numpy as np
import ml_dtypes

B = 2
N = 512
M = 2048
D = 4096
H = 4096
P = 128
MLOC = M // 4  # 512 memory rows per core
NCORES = 8

BF16 = ml_dtypes.bfloat16

_CACHE = {}


def _build_program():
    import concourse.bacc as bacc
    import concourse.mybir as mybir
    import concourse.tile as tile
    from concourse.masks import make_identity

    fp32 = mybir.dt.float32
    bf16 = mybir.dt.bfloat16

    nc = bacc.Bacc("TRN2", target_bir_lowering=False)

    KC_ = D // P
    HS_ = H // 512
    xin_t = nc.dram_tensor("xin_t", [D, N], bf16, kind="ExternalInput")
    mem_t = nc.dram_tensor("mem_t", [D, MLOC], bf16, kind="ExternalInput")
    mem_n = nc.dram_tensor("mem_n", [MLOC, D], bf16, kind="ExternalInput")
    # weights pre-tiled on host: [hs, k, 128, 512] so each (hs, k) weight
    # tile is one contiguous 128KB DMA. wq arrives as this core's 2-slice
    # share of the hidden dim (its batch group AllGathers the xq pieces).
    wq_t = nc.dram_tensor("wq_t", [2, KC_, P, 512], bf16, kind="ExternalInput")
    wk_t = nc.dram_tensor("wk_t", [HS_, KC_, P, 512], bf16, kind="ExternalInput")

    out_part = nc.dram_tensor("out_part", [N, D], fp32, kind="ExternalOutput")
    scores_raw = nc.dram_tensor("scores_raw", [N, MLOC], fp32, kind="ExternalOutput")

    KC = D // P  # 32 contraction chunks for the projections
    HS = H // 512  # 8 output column slices for the projections
    NCH = N // P  # 4 query-row chunks
    MCH = MLOC // P  # 4 memory-row chunks
    DS = D // 512  # 8 output column slices for the final matmul

    groups = [[0, 1, 2, 3], [4, 5, 6, 7]]

    with tile.TileContext(nc) as tc:
        with (
            tc.tile_pool(name="const", bufs=1) as const,
            tc.tile_pool(name="act_in", bufs=1) as act_in,
            tc.tile_pool(name="wstream", bufs=4) as wstream,
            tc.tile_pool(name="psum_acc", bufs=6, space="PSUM") as psum_acc,
            tc.tile_pool(name="psum_tr", bufs=2, space="PSUM") as psum_tr,
            tc.tile_pool(name="evict", bufs=4) as evict,
            tc.tile_pool(name="hmajor", bufs=1) as hmajor,
            tc.tile_pool(name="dram", bufs=1, space="DRAM") as dram,
            tc.tile_pool(name="soft", bufs=3) as soft,
            tc.tile_pool(name="stats", bufs=8) as stats,
            tc.tile_pool(name="outev", bufs=4) as outev,
        ):
            ident = const.tile([P, P], bf16)
            make_identity(nc, ident)

            def load_act_quad(dst, src_dram, kp):
                nc.sync.dma_start(
                    out=dst[kp][:],
                    in_=src_dram[kp * 4 * P:(kp + 1) * 4 * P, :].rearrange(
                        "(kk p) n -> p kk n", p=P
                    ),
                )

            # H-major projected activations; xqT_loc holds this core's 8
            # h-chunks (of 32) for all N
            xqT = hmajor.tile([P, KC, N], bf16, tag="xqT")
            xqT_loc = hmajor.tile([P, 8, N], bf16, tag="xqT_loc")
            mkT = hmajor.tile([P, KC, MLOC], bf16, tag="mkT")

            def project_hs(w_dram, act_sb, nch, hs, write_block, pre_dma=None):
                psums = [
                    psum_acc.tile([P, 512], fp32, tag="pacc", name="pacc")
                    for _ in range(nch)
                ]
                for kp in range(KC // 4):
                    if pre_dma is not None:
                        pre_dma(kp)
                    wt = wstream.tile([P, 4, 512], bf16, tag="wt", name="wt")
                    nc.sync.dma_start(
                        out=wt[:],
                        in_=w_dram[hs, kp * 4:(kp + 1) * 4].rearrange(
                            "kk p h -> p kk h"
                        ),
                    )
                    for kk in range(4):
                        k = kp * 4 + kk
                        for ni in range(nch):
                            nc.tensor.matmul(
                                psums[ni][:],
                                lhsT=act_sb[kp][:, kk, ni * P:(ni + 1) * P],
                                rhs=wt[:, kk, :],
                                start=(k == 0),
                                stop=(k == KC - 1),
                            )
                for ni in range(nch):
                    ev = evict.tile([P, 512], bf16, tag="ev", name="ev")
                    nc.scalar.copy(ev[:], psums[ni][:])
                    for t in range(4):
                        pt = psum_tr.tile([P, P], bf16, tag="pt", name="pt")
                        nc.tensor.transpose(
                            pt[:], ev[:, t * P:(t + 1) * P], ident[:]
                        )
                        write_block(hs * 4 + t, ni, pt)

            def write_xq(h_chunk, ni, pt):
                nc.vector.tensor_copy(
                    xqT_loc[:, h_chunk, ni * P:(ni + 1) * P], pt[:]
                )

            def write_mk(h_chunk, ni, pt):
                nc.vector.tensor_copy(mkT[:, h_chunk, ni * P:(ni + 1) * P], pt[:])

            # local xq share first (2 of 8 hidden slices, full N), then the
            # AllGather runs while the mk projection keeps PE busy. The xin
            # loads interleave with the first slice's weight stream. The
            # xin/memt/memn residents have disjoint lifetimes, so each
            # lives in its own scoped pool.
            with tc.tile_pool(name="xin_pool", bufs=1) as xin_pool:
                xin_sb = [
                    xin_pool.tile([P, 4, N], bf16, tag=f"xin{k}", name=f"xin{k}")
                    for k in range(KC // 4)
                ]
                project_hs(
                    wq_t, xin_sb, 4, 0, write_xq,
                    pre_dma=lambda kp: load_act_quad(xin_sb, xin_t, kp),
                )
                project_hs(wq_t, xin_sb, 4, 1, write_xq)

            piece = dram.tile([P, 8, N], bf16, tag="piece")
            gathered = dram.tile([4, P, 8, N], bf16, tag="gathered")
            nc.sync.dma_start(out=piece[:], in_=xqT_loc[:])
            nc.gpsimd.collective_compute(
                "AllGather",
                mybir.AluOpType.bypass,
                replica_groups=groups,
                ins=[piece[:]],
                outs=[gathered[:]],
            )
            with tc.tile_pool(name="memt_pool", bufs=1) as memt_pool:
                memt_sb = [
                    memt_pool.tile(
                        [P, 4, MLOC], bf16, tag=f"memt{k}", name=f"memt{k}"
                    )
                    for k in range(KC // 4)
                ]
                project_hs(
                    wk_t, memt_sb, 4, 0, write_mk,
                    pre_dma=lambda kp: load_act_quad(memt_sb, mem_t, kp),
                )
                for hs in range(1, HS):
                    project_hs(wk_t, memt_sb, 4, hs, write_mk)
            # pull the gathered xq pieces back after the whole weight
            # stream is queued: these DMAs wait on the collective, and
            # anything queued behind them would head-block its DMA queue
            for j in range(4):
                nc.sync.dma_start(
                    out=xqT[:, j * 8:(j + 1) * 8, :], in_=gathered[j]
                )

            import contextlib

            es = contextlib.ExitStack()
            memn_pool = es.enter_context(tc.tile_pool(name="memn_pool", bufs=1))
            memn_sb = memn_pool.tile([P, MCH, D], bf16, tag="memn")
            for mc in range(MCH):
                nc.sync.dma_start(
                    out=memn_sb[:, mc, :], in_=mem_n[mc * P:(mc + 1) * P, :]
                )

            # scores + softmax numerator + p^T
            pT = hmajor.tile([P, MCH, N], bf16, tag="pT")
            for ni in range(NCH):
                ps = psum_acc.tile([P, MLOC], fp32, tag="pacc")
                for k in range(KC):
                    nc.tensor.matmul(
                        ps[:],
                        lhsT=xqT[:, k, ni * P:(ni + 1) * P],
                        rhs=mkT[:, k, :],
                        start=(k == 0),
                        stop=(k == KC - 1),
                    )
                # The reference materializes the scores einsum in bf16, so
                # quantize raw scores to bf16 before the softmax chain; the
                # host performs the identical quantization on scores_raw.
                sc = soft.tile([P, MLOC], fp32, tag="sc")
                nc.vector.tensor_copy(sc[:], ps[:])
                nc.sync.dma_start(
                    out=scores_raw[ni * P:(ni + 1) * P, :], in_=sc[:]
                )
                sb = soft.tile([P, MLOC], bf16, tag="sb")
                nc.vector.tensor_copy(sb[:], ps[:])
                mx = stats.tile([P, 1], fp32, tag="mx")
                nc.vector.reduce_max(mx[:], sb[:], axis=mybir.AxisListType.X)
                nb = stats.tile([P, 1], fp32, tag="nb")
                nc.scalar.mul(nb[:], mx[:], -1.0 / 64.0)
                pb = soft.tile([P, MLOC], bf16, tag="pb")
                nc.scalar.activation(
                    pb[:],
                    sb[:],
                    mybir.ActivationFunctionType.Exp,
                    bias=nb[:],
                    scale=1.0 / 64.0,
                )
                for mc in range(MCH):
                    pt = psum_tr.tile([P, P], bf16)
                    nc.tensor.transpose(
                        pt[:], pb[:, mc * P:(mc + 1) * P], ident[:]
                    )
                    nc.vector.tensor_copy(
                        pT[:, mc, ni * P:(ni + 1) * P], pt[:]
                    )

            # out_part = p @ mem_slice (f32 partials)
            for ni in range(NCH):
                for dsi in range(DS):
                    po = psum_acc.tile([P, 512], fp32, tag="pacc")
                    for mc in range(MCH):
                        nc.tensor.matmul(
                            po[:],
                            lhsT=pT[:, mc, ni * P:(ni + 1) * P],
                            rhs=memn_sb[:, mc, dsi * 512:(dsi + 1) * 512],
                            start=(mc == 0),
                            stop=(mc == MCH - 1),
                        )
                    oe = outev.tile([P, 512], fp32)
                    nc.scalar.copy(oe[:], po[:])
                    nc.sync.dma_start(
                        out=out_part[ni * P:(ni + 1) * P, dsi * 512:(dsi + 1) * 512],
                        in_=oe[:],
                    )
            es.close()

    nc.compile()
    return nc


def _get_program():
    if "nc" not in _CACHE:
        _CACHE["nc"] = _build_program()
    return _CACHE["nc"]


def _as_bf16(x):
    x = np.asarray(x)
    if x.dtype != BF16:
        x = x.astype(BF16)
    return x


def make_in_maps(memory, inputs, wq, wk):
    memory = _as_bf16(memory)
    inputs = _as_bf16(inputs)
    wq = _as_bf16(wq)
    wk = _as_bf16(wk)

    def retile_w(w):
        # w is [H, D]; device wants w.T = [D, H] pre-tiled as
        # [hs, k, 128, 512] with contiguous (hs, k) tiles
        wt = np.ascontiguousarray(w.T).reshape(D // P, P, H // 512, 512)
        return np.ascontiguousarray(wt.transpose(2, 0, 1, 3))

    wq_t = retile_w(wq)
    wk_t = retile_w(wk)
    in_maps = []
    for c in range(NCORES):
        b, j = divmod(c, 4)
        mem_slice = memory[b, j * MLOC:(j + 1) * MLOC]
        in_maps.append(
            {
                "xin_t": np.ascontiguousarray(inputs[b].T),
                "wq_t": np.ascontiguousarray(wq_t[2 * j:2 * j + 2]),
                "mem_t": np.ascontiguousarray(mem_slice.T),
                "mem_n": np.ascontiguousarray(mem_slice),
                "wk_t": wk_t,
            }
        )
    return in_maps


def kernel(memory, inputs, wq, wk, seg_num, _want_results=False):
    from concourse.bass_utils import run_bass_kernel_spmd

    seg = int(np.asarray(seg_num))
    nc = _get_program()
    in_maps = make_in_maps(memory, inputs, wq, wk)
    res = run_bass_kernel_spmd(nc, in_maps, list(range(NCORES)))

    output = np.empty((B, N, D), dtype=np.float32)
    hist_parts = []
    for b in range(B):
        raw = np.concatenate(
            [res.results[4 * b + j]["scores_raw"] for j in range(4)], axis=1
        ).astype(np.float32)  # [N, M]
        # mirror the reference: the scores einsum materializes in bf16
        logits = (raw * np.float32(1.0 / 64.0)).astype(BF16).astype(np.float32)
        m_glob = logits.max(axis=1)
        l_glob = np.exp(logits - m_glob[:, None]).sum(axis=1)
        acc = np.zeros((N, D), dtype=np.float32)
        for j in range(4):
            m_loc = logits[:, j * MLOC:(j + 1) * MLOC].max(axis=1)
            scale = np.exp(m_loc - m_glob)
            acc += res.results[4 * b + j]["out_part"] * scale[:, None]
        output[b] = acc / l_glob[:, None]
        am = np.argmax(logits, axis=1)
        hist_parts.append((seg - am).astype(np.int32))

    hist = np.concatenate(hist_parts)
    browse = np.bool_(hist[0] < 4)
    out = (output.astype(BF16), hist, browse)
    if _want_results:
        return out, res
    return out
